# revision 1
# baseline (speedup 1.0000x reference)
"""DDiT block (adaLN transformer block) on 8 Trainium2 NeuronCores.

Sharding: sequence-parallel everywhere + per-batch K/V AllGather (Ulysses-style).
  - 8 cores = 2 batch groups x 4 sequence ranks. Core g handles batch b=g//4,
    rows [r*512, (r+1)*512) with r=g%4.
  - Each core: adaLN mod (replicated per batch group), LN1+modulate on its rows,
    QKV projection for its rows (all heads), RoPE, AllGather of K^T and V across
    its group of 4, full attention for its 512 query rows x 16 heads, out-proj,
    residual, LN2+modulate, full MLP on its rows, residual.
  - No reduction collectives: every core produces complete output rows.

Layout: activations are kept transposed ("T-layout", feature-on-partition,
sequence-on-free) so every projection matmul has its contraction dim on
partitions with weights in natural layout. The attention AV product consumes
softmax(scores) transposed, which is exactly what the scoresT layout produces;
the softmax denominator comes from an appended ones-column in V.

All matmul operands are bf16 (weights pre-cast on host); accumulation fp32;
LayerNorm statistics and residual stream kept in fp32.
"""
import os
import sys

sys.path.insert(0, "/opt/trn_rl_repo")

import numpy as np
import ml_dtypes

import concourse.bass as bass
import concourse.mybir as mybir
import concourse.tile as tile
from concourse.bass_utils import run_bass_kernel_spmd
from concourse.masks import make_identity
from concourse.vector_clock import ScopedClock
import bass_rust

BF = mybir.dt.bfloat16
F32 = mybir.dt.float32
AF = mybir.ActivationFunctionType
OP = mybir.AluOpType

B, S, D, H, HD, COND, MLP_H = 2, 2048, 1024, 16, 64, 1024, 4096
G = 4              # ranks per batch group
SL = S // G        # 512 local rows
EPS = 1e-6
RG = [[0, 1, 2, 3], [4, 5, 6, 7]]
DEBUG = bool(int(os.environ.get("KBENCH_DEBUG", "0")))


def _patched_drain_and_barrier(self, tick_clock, wait_clock):
    # This build's rust layer allows only one sem wait per instruction; stock
    # TileContext crams every final wait onto a single Drain, which walrus
    # rejects ("Too many sync wait commands"). Spread them over nops.
    nc = self.nc
    probe = nc.sync.nop(nofuse=True)
    wait_clock.add_sem_waits(probe.ins, ScopedClock({None: tick_clock.global_clock}))
    waits = list(probe.ins.sync_info.on_wait)
    probe.ins.sync_info.on_wait = waits[:1]
    for w in waits[1:]:
        n2 = nc.sync.nop(nofuse=True)
        n2.ins.sync_info = bass_rust.SyncInfo(on_wait=[w], on_update=[])
    nc.sync.drain()
    nc.all_engine_barrier()
    assert self.sems is not None
    popped = nc._tile_sem_poison_stack.pop()
    assert popped is self._sem_poison
    nc.clear_and_free_semaphores(list(self.sems.allocated().values()))
    nc.all_engine_barrier()


tile.TileContext._drain_and_barrier = _patched_drain_and_barrier

_orig_to_json_bytes = bass.Bass.to_json_bytes


def _to_json_bytes_split_waits(self):
    """This walrus build accepts at most one sem wait per instruction, but
    Tile's sem assignment attaches several. Spill excess waits onto freshly
    inserted EventSemaphore instructions on the same engine, just before the
    over-committed instruction (per-engine program order preserved)."""
    import json as _json
    d = _json.loads(_orig_to_json_bytes(self))
    ctr = 0
    for f in d.get("functions", []):
        for blk in f.get("blocks", []):
            out = []
            for inst in blk.get("instructions", []):
                si = inst.get("sync_info")
                waits = (si or {}).get("on_wait") or []
                if len(waits) > 1:
                    for w in waits[:-1]:
                        ctr += 1
                        ev = {
                            "engine": inst.get("engine"),
                            "ins": [],
                            "name": f"evsplit_{ctr}",
                            "opcode": "EventSemaphore",
                            "outs": [],
                            "sync_info": {"on_update": [], "on_wait": [w]},
                        }
                        if "debug" in inst:
                            ev["debug"] = inst["debug"]
                        out.append(ev)
                    si["on_wait"] = waits[-1:]
                out.append(inst)
            blk["instructions"] = out
    return _json.dumps(d).encode()


bass.Bass.to_json_bytes = _to_json_bytes_split_waits


def build():
    nc = bass.Bass(num_devices=8)

    # ---- I/O ----
    xT = nc.dram_tensor("xT", [D, SL], F32, kind="ExternalInput")
    cT = nc.dram_tensor("cT", [128, COND // 128], BF, kind="ExternalInput")
    bada = nc.dram_tensor("bada", [1, 3, 512], BF, kind="ExternalInput")
    cosdT = nc.dram_tensor("cosdT", [128, SL], F32, kind="ExternalInput")
    sindT = nc.dram_tensor("sindT", [128, SL], F32, kind="ExternalInput")
    pswap = nc.dram_tensor("pswap", [128, 128], BF, kind="ExternalInput")
    wqkv = nc.dram_tensor("wqkv", [D, 3 * D], BF, kind="ExternalInput")
    wout = nc.dram_tensor("wout", [D, D], BF, kind="ExternalInput")
    w1 = nc.dram_tensor("w1", [D, MLP_H], BF, kind="ExternalInput")
    w2 = nc.dram_tensor("w2", [MLP_H, D], BF, kind="ExternalInput")
    wada = nc.dram_tensor("wada", [COND, (6 * D) // G], BF, kind="ExternalInput")
    yT = nc.dram_tensor("yT", [D, SL], F32, kind="ExternalOutput")

    dbg = {}
    if DEBUG:
        dbg["mod"] = nc.dram_tensor("dbg_mod", [128, 48], F32, kind="ExternalOutput")
        dbg["xn1"] = nc.dram_tensor("dbg_xn1", [128, 8, SL], BF, kind="ExternalOutput")
        dbg["q"] = nc.dram_tensor("dbg_q", [64, 16, SL], BF, kind="ExternalOutput")
        dbg["attn"] = nc.dram_tensor("dbg_attn", [128, 8, SL], BF, kind="ExternalOutput")
        dbg["x2"] = nc.dram_tensor("dbg_x2", [128, 8, SL], F32, kind="ExternalOutput")
        dbg["agk"] = nc.dram_tensor("dbg_agk", [G * D, SL], BF, kind="ExternalOutput")

    wqkv_r = wqkv[:].rearrange("(ko p) f -> p ko f", p=128)      # [128, 8, 3072]
    wout_r = wout[:].rearrange("(ko p) f -> p ko f", p=128)      # [128, 8, 1024]
    w1_r = w1[:].rearrange("(ko p) f -> p ko f", p=128)          # [128, 8, 4096]
    w2_r = w2[:].rearrange("(kt p) f -> p kt f", p=128)          # [128, 32, 1024]
    wada_r = wada[:].rearrange("(ko p) f -> p ko f", p=128)      # [128, 8, 1536]
    xT_r = xT[:].rearrange("(ko p) s -> p ko s", p=128)          # [128, 8, 512]
    yT_r = yT[:].rearrange("(ko p) s -> p ko s", p=128)

    with tile.TileContext(nc) as tc:
        with (
            tc.tile_pool(name="pp", bufs=1) as pp,
            tc.tile_pool(name="tmp_ln", bufs=1) as tmp_ln,
            tc.tile_pool(name="scratch", bufs=4) as scratch,
            tc.tile_pool(name="rows", bufs=2) as rows,
            tc.tile_pool(name="rope", bufs=3) as rope_p,
            tc.tile_pool(name="ps_mm", bufs=3, space="PSUM") as ps_mm,
            tc.tile_pool(name="ps_stat", bufs=1, space="PSUM") as ps_stat,
            tc.tile_pool(name="ps_aux", bufs=2, space="PSUM") as ps_aux,
            tc.tile_pool(name="ps_avtr", bufs=2, space="PSUM") as ps_avtr,
            tc.tile_pool(name="dram", bufs=1, space="DRAM") as dram,
        ):
            # ---- global constants & residual-stream tensors ----
            one1_bf = pp.tile([1, 1], BF, tag="one1")
            nc.vector.memset(one1_bf[:], 1.0)
            ones128_bf = pp.tile([128, 1], BF, tag="ones128")
            nc.vector.memset(ones128_bf[:], 1.0)
            ones1x128_f = pp.tile([1, 128], F32, tag="ones1x128")
            nc.vector.memset(ones1x128_f[:], 1.0)
            ones1x64_bf = pp.tile([1, 64], BF, tag="ones1x64")
            nc.vector.memset(ones1x64_bf[:], 1.0)
            eps_sb = pp.tile([1, 1], F32, tag="eps")
            nc.vector.memset(eps_sb[:], EPS)

            xT_sb = pp.tile([128, 8, SL], F32, tag="xT")
            for i in range(4):
                nc.sync.dma_start(xT_sb[:, 2 * i:2 * i + 2, :], xT_r[:, 2 * i:2 * i + 2, :])
            x2T = pp.tile([128, 8, SL], F32, tag="x2T")
            mod_T = pp.tile([128, 48], F32, tag="modT")
            sc1p_msa = pp.tile([128, 8], F32, tag="sc1p_msa")
            sc1p_mlp = pp.tile([128, 8], F32, tag="sc1p_mlp")

            def scr():
                return scratch.tile([128, SL], F32, tag="scratch", name="scr")

            # ---- LayerNorm in T-layout (stats via ones-matmul) + modulate ----
            def layernorm_mod(src_sb, sc1p, sh_col0, xn):
                xbf = tmp_ln.tile([128, 8, SL], BF, tag="lnbf", name="xbf")
                nc.vector.tensor_copy(xbf[:], src_sb[:])
                x2bf = tmp_ln.tile([128, 8, SL], BF, tag="lnbf2", name="x2bf")
                nc.vector.tensor_tensor(x2bf[:], src_sb[:], src_sb[:], OP.mult)
                ps_sum = ps_stat.tile([1, SL], F32, tag="stat", name="ps_sum")
                for ko in range(8):
                    nc.tensor.matmul(ps_sum[:], ones128_bf[:], xbf[:, ko, :],
                                     start=(ko == 0), stop=(ko == 7))
                m_sb = rows.tile([1, SL], F32, tag="m", name="m_sb")
                nc.vector.tensor_scalar_mul(m_sb[:], ps_sum[:], 1.0 / D)
                ps_sq = ps_stat.tile([1, SL], F32, tag="stat", name="ps_sq")
                for ko in range(8):
                    nc.tensor.matmul(ps_sq[:], ones128_bf[:], x2bf[:, ko, :],
                                     start=(ko == 0), stop=(ko == 7))
                var_sb = rows.tile([1, SL], F32, tag="var", name="var_sb")
                nc.vector.tensor_scalar_mul(var_sb[:], ps_sq[:], 1.0 / D)
                m2_sb = rows.tile([1, SL], F32, tag="m2", name="m2_sb")
                nc.vector.tensor_tensor(m2_sb[:], m_sb[:], m_sb[:], OP.mult)
                nc.vector.tensor_tensor(var_sb[:], var_sb[:], m2_sb[:], OP.subtract)
                rstd_sb = rows.tile([1, SL], F32, tag="rstd", name="rstd_sb")
                nc.scalar.activation(rstd_sb[:], var_sb[:], AF.Sqrt,
                                     bias=eps_sb[:], scale=1.0)
                nc.vector.reciprocal(rstd_sb[:], rstd_sb[:])
                m_rep = ps_aux.tile([128, SL], F32, tag="aux", name="m_rep")
                nc.tensor.matmul(m_rep[:], ones1x128_f[:], m_sb[:], start=True, stop=True)
                rstd_rep = ps_aux.tile([128, SL], F32, tag="aux", name="rstd_rep")
                nc.tensor.matmul(rstd_rep[:], ones1x128_f[:], rstd_sb[:],
                                 start=True, stop=True)
                for ko in range(8):
                    t1 = scr()
                    nc.vector.tensor_tensor(t1[:], src_sb[:, ko, :], m_rep[:], OP.subtract)
                    t2 = scr()
                    nc.vector.tensor_tensor(t2[:], t1[:], rstd_rep[:], OP.mult)
                    nc.vector.tensor_scalar(
                        xn[:, ko, :], t2[:],
                        scalar1=sc1p[:, ko:ko + 1],
                        scalar2=mod_T[:, sh_col0 + ko:sh_col0 + ko + 1],
                        op0=OP.mult, op1=OP.add)

            with tc.tile_pool(name="q64p", bufs=1) as q64p:
                q64 = q64p.tile([64, 16, SL], BF, tag="q64")

                with (
                    tc.tile_pool(name="qscope", bufs=1) as qs,
                    tc.tile_pool(name="wada_p", bufs=2) as wada_p,
                    tc.tile_pool(name="wqkv_p", bufs=1) as wqkv_p,
                ):
                    ident = qs.tile([128, 128], BF, tag="ident")
                    make_identity(nc, ident[:])
                    pswap_sb = qs.tile([128, 128], BF, tag="pswap")
                    nc.sync.dma_start(pswap_sb[:], pswap[:])
                    cosd_sb = qs.tile([128, SL], F32, tag="cosd")
                    nc.sync.dma_start(cosd_sb[:], cosdT[:])
                    sind_sb = qs.tile([128, SL], F32, tag="sind")
                    nc.sync.dma_start(sind_sb[:], sindT[:])
                    cT_sb = qs.tile([128, 8], BF, tag="cT")
                    nc.sync.dma_start(cT_sb[:], cT[:])
                    bada_sb = qs.tile([1, 3, 512], BF, tag="bada")
                    nc.sync.dma_start(bada_sb[:], bada[:])

                    # ---- adaLN modulation: each rank computes 1/4 of mod,
                    # AllGather reassembles, then D-on-partition readback ----
                    ag_m_in = dram.tile([1, (6 * D) // G], F32)
                    ag_m_out = dram.tile([G, (6 * D) // G], F32)
                    for j in range(3):
                        wada_t = wada_p.tile([128, 8, 512], BF, tag="wada", name="wada_t")
                        nc.sync.dma_start(wada_t[:], wada_r[:, :, j * 512:(j + 1) * 512])
                        st = ps_stat.tile([1, 512], F32, tag="stat", name="st_mod")
                        for ko in range(8):
                            nc.tensor.matmul(st[:], cT_sb[:, ko:ko + 1], wada_t[:, ko, :],
                                             start=(ko == 0), stop=False)
                        nc.tensor.matmul(st[:], one1_bf[:], bada_sb[0:1, j, :],
                                         start=False, stop=True)
                        row = rows.tile([1, 512], F32, tag="modrow", name="modrow")
                        nc.vector.tensor_copy(row[:], st[:])
                        nc.sync.dma_start(ag_m_in[0:1, j * 512:(j + 1) * 512], row[:])
                    nc.gpsimd.collective_compute(
                        "AllGather", OP.bypass, replica_groups=RG,
                        ins=[ag_m_in.opt()], outs=[ag_m_out.opt()])
                    nc.sync.dma_start(
                        mod_T[:],
                        ag_m_out[:].rearrange("r (o p) -> p (r o)", p=128))
                    nc.vector.tensor_scalar_add(sc1p_msa[:], mod_T[:, 8:16], 1.0)
                    nc.vector.tensor_scalar_add(sc1p_mlp[:], mod_T[:, 32:40], 1.0)
                    if DEBUG:
                        nc.sync.dma_start(dbg["mod"][:], mod_T[:])

                    xn1 = qs.tile([128, 8, SL], BF, tag="xn1")
                    layernorm_mod(xT_sb, sc1p_msa, 0, xn1)
                    if DEBUG:
                        nc.sync.dma_start(dbg["xn1"][:], xn1[:])

                    # ---- QKV (transposed) + RoPE + AllGathers ----
                    wqkv_sb = wqkv_p.tile([128, 8, 3 * D], BF, tag="wqkv")
                    for ko in range(8):
                        nc.sync.dma_start(wqkv_sb[:, ko, :], wqkv_r[:, ko, :])
                    ag_k_in = dram.tile([D, SL], BF)
                    ag_k_out = dram.tile([G * D, SL], BF)
                    ag_v_in = dram.tile([SL, H * 65], BF)
                    ag_v_out = dram.tile([G * SL, H * 65], BF)
                    v_aug = qs.tile([128, 4, H, 65], BF, tag="v_aug")
                    nc.vector.memset(v_aug[:, :, :, 64:65], 1.0)

                    def rope_chunk(ps):
                        raw = rope_p.tile([128, SL], BF, tag="raw", name="raw")
                        nc.vector.tensor_copy(raw[:], ps[:])
                        swp = ps_avtr.tile([128, SL], F32, tag="avtr", name="swp")
                        nc.tensor.matmul(swp[:], pswap_sb[:], raw[:], start=True, stop=True)
                        t1 = scr()
                        nc.vector.tensor_tensor(t1[:], swp[:], sind_sb[:], OP.mult)
                        t2 = scr()
                        nc.vector.tensor_tensor(t2[:], raw[:], cosd_sb[:], OP.mult)
                        dst = rope_p.tile([128, SL], BF, tag="dst", name="dst")
                        nc.vector.tensor_tensor(dst[:], t1[:], t2[:], OP.add)
                        return dst

                    for nt in (2, 3, 4, 5, 0, 1):  # k first, then v, then q
                        for mc in range(4):
                            fc = nt * 4 + mc
                            ps = ps_mm.tile([128, SL], F32, tag="mm", name="ps_qkv")
                            for ko in range(8):
                                nc.tensor.matmul(
                                    ps[:], wqkv_sb[:, ko, fc * 128:(fc + 1) * 128],
                                    xn1[:, ko, :], start=(ko == 0), stop=(ko == 7))
                            dst = rope_chunk(ps)
                            if 8 <= fc < 16:        # k chunk -> AG input (T-layout)
                                r0 = (fc - 8) * 128
                                nc.sync.dma_start(ag_k_in[r0:r0 + 128, :], dst[:])
                            elif fc >= 16:          # v chunk -> s-layout + ones col
                                hv = (fc - 16) * 2
                                for si in range(4):
                                    tp = ps_avtr.tile([128, 128], BF, tag="avtr", name="tp")
                                    nc.tensor.transpose(
                                        tp[:], dst[:, si * 128:(si + 1) * 128], ident[:])
                                    nc.vector.tensor_copy(v_aug[:, si, hv, 0:64],
                                                          tp[:, 0:64])
                                    nc.vector.tensor_copy(v_aug[:, si, hv + 1, 0:64],
                                                          tp[:, 64:128])
                            else:                   # q chunk stays local (per-head)
                                nc.sync.dma_start(q64[:, 2 * fc, :], dst[0:64, :])
                                nc.sync.dma_start(q64[:, 2 * fc + 1, :], dst[64:128, :])
                        if nt == 3:
                            nc.gpsimd.collective_compute(
                                "AllGather", OP.bypass, replica_groups=RG,
                                ins=[ag_k_in.opt()], outs=[ag_k_out.opt()])
                        if nt == 5:
                            nc.sync.dma_start(
                                ag_v_in[:].rearrange("(si p) (h w) -> p si h w",
                                                     p=128, h=H),
                                v_aug[:])
                            nc.gpsimd.collective_compute(
                                "AllGather", OP.bypass, replica_groups=RG,
                                ins=[ag_v_in.opt()], outs=[ag_v_out.opt()])

                    if DEBUG:
                        nc.sync.dma_start(dbg["q"][:], q64[:])
                        nc.sync.dma_start(dbg["agk"][:], ag_k_out[:])

                # ---- attention ----
                with (
                    tc.tile_pool(name="ascope", bufs=1) as asc,
                    tc.tile_pool(name="kth_p", bufs=2) as kth_p,
                    tc.tile_pool(name="exph_p", bufs=1) as exph_p,
                    tc.tile_pool(name="wout_p", bufs=3) as wout_p,
                ):
                    v_full = asc.tile([128, 16, H, 65], BF, tag="v_full")
                    for r in range(G):
                        nc.sync.dma_start(
                            v_full[:, r * 4:(r + 1) * 4, :, :],
                            ag_v_out[r * SL:(r + 1) * SL, :].rearrange(
                                "(si p) (h w) -> p si h w", p=128, h=H))
                    attn64 = asc.tile([64, 16, SL], BF, tag="attn64")
                    for h in range(H):
                        kT_h = kth_p.tile([64, G, SL], BF, tag="kth", name="kT_h")
                        for r in range(G):
                            nc.sync.dma_start(
                                kT_h[:, r, :],
                                ag_k_out[r * D + h * 64:r * D + (h + 1) * 64, :])
                        exp_h = exph_p.tile([128, 16, SL], BF, tag="exph", name="exp_h")
                        for m in range(16):
                            sc_ps = ps_mm.tile([128, SL], F32, tag="mm", name="sc_ps")
                            nc.tensor.matmul(
                                sc_ps[:],
                                kT_h[:, m // 4, (m % 4) * 128:(m % 4) * 128 + 128],
                                q64[:, h, :], start=True, stop=True)
                            nc.scalar.activation(exp_h[:, m, :], sc_ps[:], AF.Exp,
                                                 scale=1.0 / float(np.sqrt(HD)))
                        av = ps_avtr.tile([65, SL], F32, tag="avtr", name="av")
                        for m in range(16):
                            nc.tensor.matmul(av[:], v_full[:, m, h, :], exp_h[:, m, :],
                                             start=(m == 0), stop=(m == 15))
                        rec_f = rows.tile([1, SL], F32, tag="recf", name="rec_f")
                        nc.vector.reciprocal(rec_f[:], av[64:65, :])
                        rec = rows.tile([1, SL], BF, tag="rec", name="rec")
                        nc.vector.tensor_copy(rec[:], rec_f[:])
                        rec_rep = ps_aux.tile([64, SL], F32, tag="aux", name="rec_rep")
                        nc.tensor.matmul(rec_rep[:], ones1x64_bf[:], rec[:],
                                         start=True, stop=True)
                        rec_rep_sb = rope_p.tile([64, SL], F32, tag="recrep", name="rec_rep_sb")
                        nc.vector.tensor_copy(rec_rep_sb[:], rec_rep[:])
                        nc.vector.tensor_tensor(attn64[:, h, :], av[0:64, :],
                                                rec_rep_sb[:], OP.mult)

                    attn_sb = asc.tile([128, 8, SL], BF, tag="attn")
                    for fc in range(8):
                        nc.sync.dma_start(attn_sb[0:64, fc, :], attn64[:, 2 * fc, :])
                        nc.sync.dma_start(attn_sb[64:128, fc, :], attn64[:, 2 * fc + 1, :])
                    if DEBUG:
                        nc.sync.dma_start(dbg["attn"][:], attn_sb[:])

                    # ---- out projection + gated residual ----
                    for dc in range(8):
                        wo_t = wout_p.tile([128, 8, 128], BF, tag="wo", name="wo_t")
                        nc.sync.dma_start(wo_t[:], wout_r[:, :, dc * 128:(dc + 1) * 128])
                        ps = ps_mm.tile([128, SL], F32, tag="mm", name="ps_out")
                        for ko in range(8):
                            nc.tensor.matmul(ps[:], wo_t[:, ko, :], attn_sb[:, ko, :],
                                             start=(ko == 0), stop=(ko == 7))
                        tg = scr()
                        nc.vector.tensor_scalar_mul(tg[:], ps[:],
                                                    mod_T[:, 16 + dc:17 + dc])
                        nc.vector.tensor_tensor(x2T[:, dc, :], xT_sb[:, dc, :],
                                                tg[:], OP.add)
                    if DEBUG:
                        nc.sync.dma_start(dbg["x2"][:], x2T[:])

            # ---- LN2 + MLP ----
            with tc.tile_pool(name="hT_p", bufs=1) as hT_p:
                hT = hT_p.tile([128, 32, SL], BF, tag="hT")
                with (
                    tc.tile_pool(name="m1scope", bufs=1) as m1s,
                    tc.tile_pool(name="w1_p", bufs=1) as w1_p,
                ):
                    xn2 = m1s.tile([128, 8, SL], BF, tag="xn2")
                    layernorm_mod(x2T, sc1p_mlp, 24, xn2)
                    w1_sb = w1_p.tile([128, 8, MLP_H], BF, tag="w1")
                    for ko in range(8):
                        nc.sync.dma_start(w1_sb[:, ko, :], w1_r[:, ko, :])
                    for mt in range(32):
                        ps = ps_mm.tile([128, SL], F32, tag="mm", name="ps_m1")
                        for ko in range(8):
                            nc.tensor.matmul(
                                ps[:], w1_sb[:, ko, mt * 128:(mt + 1) * 128],
                                xn2[:, ko, :], start=(ko == 0), stop=(ko == 7))
                        nc.scalar.activation(hT[:, mt, :], ps[:], AF.Gelu_apprx_tanh)

                with tc.tile_pool(name="w2_p", bufs=2) as w2_p:
                    for dc in range(8):
                        ps = ps_mm.tile([128, SL], F32, tag="mm", name="ps_m2")
                        for kq in range(4):
                            w2_t = w2_p.tile([128, 8, 128], BF, tag="w2", name="w2_t")
                            nc.sync.dma_start(
                                w2_t[:],
                                w2_r[:, kq * 8:(kq + 1) * 8, dc * 128:(dc + 1) * 128])
                            for kk in range(8):
                                kt = kq * 8 + kk
                                nc.tensor.matmul(ps[:], w2_t[:, kk, :], hT[:, kt, :],
                                                 start=(kt == 0), stop=(kt == 31))
                        tg = scr()
                        nc.vector.tensor_scalar_mul(tg[:], ps[:],
                                                    mod_T[:, 40 + dc:41 + dc])
                        nc.vector.tensor_tensor(x2T[:, dc, :], x2T[:, dc, :],
                                                tg[:], OP.add)

            for i in range(4):
                nc.sync.dma_start(yT_r[:, 2 * i:2 * i + 2, :], x2T[:, 2 * i:2 * i + 2, :])

    return nc


_NC_CACHE = None


def _prep_in_maps(inputs):
    x = np.asarray(inputs["x"], dtype=np.float32)
    c = np.asarray(inputs["c"], dtype=np.float32)
    cos = np.asarray(inputs["cos"], dtype=np.float32)
    sin = np.asarray(inputs["sin"], dtype=np.float32)

    def b16(a):
        return np.ascontiguousarray(a).astype(ml_dtypes.bfloat16)

    wqkv_b = b16(inputs["W_qkv"])
    wout_b = b16(inputs["W_out"])
    w1_b = b16(inputs["W1"])
    w2_b = b16(inputs["W2"])
    wada_full = b16(inputs["W_ada"])
    bada_full = b16(np.asarray(inputs["b_ada"], dtype=np.float32).reshape(1, 12, 512))

    jj = np.arange(128) % 64
    pair = jj // 2
    sign = np.where(jj % 2 == 0, -1.0, 1.0).astype(np.float32)
    pswap_m = np.zeros((128, 128), np.float32)
    pswap_m[np.arange(128) ^ 1, np.arange(128)] = 1.0
    pswap_m = b16(pswap_m)

    in_maps = []
    for g in range(8):
        b, r = g // G, g % G
        rows = slice(r * SL, (r + 1) * SL)
        cl = cos[rows, 0:HD // 2]     # [512, 32]
        sl = sin[rows, 0:HD // 2]
        cosdT_m = np.ascontiguousarray(cl.T[pair])             # [128, 512]
        sindT_m = np.ascontiguousarray(sl.T[pair] * sign[:, None])
        in_maps.append({
            "xT": np.ascontiguousarray(x[b, rows, :].T),
            "cT": b16(c[b].reshape(8, 128).T),
            "bada": np.ascontiguousarray(bada_full[:, 3 * r:3 * (r + 1), :]),
            "cosdT": cosdT_m,
            "sindT": sindT_m,
            "pswap": pswap_m,
            "wqkv": wqkv_b, "wout": wout_b, "w1": w1_b, "w2": w2_b,
            "wada": np.ascontiguousarray(wada_full[:, 1536 * r:1536 * (r + 1)]),
        })
    return in_maps


LAST_RESULT = None


def kernel(**inputs) -> np.ndarray:
    global _NC_CACHE, LAST_RESULT
    if _NC_CACHE is None:
        _NC_CACHE = build()
    nc = _NC_CACHE
    in_maps = _prep_in_maps(inputs)
    res = run_bass_kernel_spmd(nc, in_maps, core_ids=list(range(8)))
    LAST_RESULT = res
    y = np.empty((B, S, D), np.float32)
    for g in range(8):
        b, r = g // G, g % G
        y[b, r * SL:(r + 1) * SL, :] = res.results[g]["yT"].T
    return y



# revision 11
# speedup vs baseline: 1.0245x; 1.0245x over previous
"""DDiT block (adaLN transformer block) on 8 Trainium2 NeuronCores.

Sharding: sequence-parallel everywhere + per-batch K/V AllGather (Ulysses-style).
  - 8 cores = 2 batch groups x 4 sequence ranks. Core g handles batch b=g//4,
    rows [r*512, (r+1)*512) with r=g%4.
  - adaLN mod: the msa sh/sc half (cols 0..2047) is computed replicated on
    every core (no collective on the critical path); the rest (g_msa + mlp
    mods, cols 2048..6143) is computed 1/4-sharded and AllGathered (the AG
    rides out the one-time CC bootstrap barrier).
  - QKV projection in fp8 DoubleRow (weights pre-scaled x16 on host), RoPE in
    bf16, K^T / V AllGathers carry fp8 payloads, attention scores in fp8,
    softmax exp on ACT with fused bias (no max-subtraction; constant folded
    into exp cancels in the denominator), AV + out-proj in fp8 DoubleRow.
  - MLP stays bf16 (the adaLN gates are ~0.64 rms so the MLP branch carries
    ~16% of the output energy; fp8 there costs ~1.5e-2 rel err).

Layout: activations are kept transposed ("T-layout", feature-on-partition,
sequence-on-free). The attention AV product consumes softmax(scores)
transposed; the softmax denominator comes from an appended ones-column in V.
"""
import os
import sys

sys.path.insert(0, "/opt/trn_rl_repo")

import numpy as np
import ml_dtypes

import concourse.bass as bass
import concourse.mybir as mybir
import concourse.tile as tile
from concourse.bass_utils import run_bass_kernel_spmd
from concourse.masks import make_identity
from concourse.vector_clock import ScopedClock
import bass_rust

BF = mybir.dt.bfloat16
F32 = mybir.dt.float32
F8 = mybir.dt.float8e4
AF = mybir.ActivationFunctionType
OP = mybir.AluOpType
DR = mybir.MatmulPerfMode.DoubleRow

B, S, D, H, HD, COND, MLP_H = 2, 2048, 1024, 16, 64, 1024, 4096
G = 4              # ranks per batch group
SL = S // G        # 512 local rows
EPS = 1e-6
WS = 16.0          # host-side prescale on fp8 weights (wqkv, wout)
RG = [[0, 1, 2, 3], [4, 5, 6, 7]]
DEBUG = bool(int(os.environ.get("KBENCH_DEBUG", "0")))


def _patched_drain_and_barrier(self, tick_clock, wait_clock):
    # This build's rust layer allows only one sem wait per instruction; stock
    # TileContext crams every final wait onto a single Drain, which walrus
    # rejects ("Too many sync wait commands"). Spread them over nops.
    nc = self.nc
    probe = nc.sync.nop(nofuse=True)
    wait_clock.add_sem_waits(probe.ins, ScopedClock({None: tick_clock.global_clock}))
    waits = list(probe.ins.sync_info.on_wait)
    probe.ins.sync_info.on_wait = waits[:1]
    for w in waits[1:]:
        n2 = nc.sync.nop(nofuse=True)
        n2.ins.sync_info = bass_rust.SyncInfo(on_wait=[w], on_update=[])
    nc.sync.drain()
    nc.all_engine_barrier()
    assert self.sems is not None
    popped = nc._tile_sem_poison_stack.pop()
    assert popped is self._sem_poison
    nc.clear_and_free_semaphores(list(self.sems.allocated().values()))
    nc.all_engine_barrier()


tile.TileContext._drain_and_barrier = _patched_drain_and_barrier

_orig_to_json_bytes = bass.Bass.to_json_bytes


def _to_json_bytes_split_waits(self):
    """This walrus build accepts at most one sem wait per instruction, but
    Tile's sem assignment attaches several. Spill excess waits onto freshly
    inserted EventSemaphore instructions on the same engine, just before the
    over-committed instruction (per-engine program order preserved)."""
    import json as _json
    d = _json.loads(_orig_to_json_bytes(self))
    ctr = 0
    for f in d.get("functions", []):
        for blk in f.get("blocks", []):
            out = []
            for inst in blk.get("instructions", []):
                si = inst.get("sync_info")
                waits = (si or {}).get("on_wait") or []
                if len(waits) > 1:
                    for w in waits[:-1]:
                        ctr += 1
                        ev = {
                            "engine": inst.get("engine"),
                            "ins": [],
                            "name": f"evsplit_{ctr}",
                            "opcode": "EventSemaphore",
                            "outs": [],
                            "sync_info": {"on_update": [], "on_wait": [w]},
                        }
                        if "debug" in inst:
                            ev["debug"] = inst["debug"]
                        out.append(ev)
                    si["on_wait"] = waits[-1:]
                out.append(inst)
            blk["instructions"] = out
    return _json.dumps(d).encode()


bass.Bass.to_json_bytes = _to_json_bytes_split_waits


def build():
    nc = bass.Bass(num_devices=8)

    # ---- I/O ----
    xT = nc.dram_tensor("xT", [D, SL], F32, kind="ExternalInput")
    cT = nc.dram_tensor("cT", [128, COND // 128], BF, kind="ExternalInput")
    bada_loc = nc.dram_tensor("bada_loc", [1, 4, 512], BF, kind="ExternalInput")
    bada_sh = nc.dram_tensor("bada_sh", [1, 2, 512], BF, kind="ExternalInput")
    cosdT = nc.dram_tensor("cosdT", [128, SL], BF, kind="ExternalInput")
    sindT = nc.dram_tensor("sindT", [128, SL], BF, kind="ExternalInput")
    pswap = nc.dram_tensor("pswap", [128, 128], BF, kind="ExternalInput")
    wqkv = nc.dram_tensor("wqkv", [D, 3 * D], F8, kind="ExternalInput")
    wout = nc.dram_tensor("wout", [D, D], F8, kind="ExternalInput")
    w1 = nc.dram_tensor("w1", [D, MLP_H], BF, kind="ExternalInput")
    w2 = nc.dram_tensor("w2", [MLP_H, D], BF, kind="ExternalInput")
    wada_loc = nc.dram_tensor("wada_loc", [COND, 2048], BF, kind="ExternalInput")
    wada_sh = nc.dram_tensor("wada_sh", [COND, 1024], BF, kind="ExternalInput")
    yT = nc.dram_tensor("yT", [D, SL], F32, kind="ExternalOutput")

    dbg = {}
    if DEBUG:
        dbg["mod"] = nc.dram_tensor("dbg_mod", [128, 48], F32, kind="ExternalOutput")
        dbg["xn1"] = nc.dram_tensor("dbg_xn1", [128, 8, SL], F8, kind="ExternalOutput")
        dbg["q"] = nc.dram_tensor("dbg_q", [64, 16, SL], F8, kind="ExternalOutput")
        dbg["attn"] = nc.dram_tensor("dbg_attn", [128, 8, SL], F8, kind="ExternalOutput")
        dbg["x2"] = nc.dram_tensor("dbg_x2", [128, 8, SL], F32, kind="ExternalOutput")
        dbg["agk"] = nc.dram_tensor("dbg_agk", [G * D, SL], F8, kind="ExternalOutput")

    wqkv_r = wqkv[:].rearrange("(ko p) f -> p ko f", p=128)        # [128, 8, 3072]
    wout_r = wout[:].rearrange("(ko p) f -> p ko f", p=128)        # [128, 8, 1024]
    w1_r = w1[:].rearrange("(ko p) f -> p ko f", p=128)            # [128, 8, 4096]
    w2_r = w2[:].rearrange("(kt p) f -> p kt f", p=128)            # [128, 32, 1024]
    wada_loc_r = wada_loc[:].rearrange("(ko p) f -> p ko f", p=128)  # [128, 8, 2048]
    wada_sh_r = wada_sh[:].rearrange("(ko p) f -> p ko f", p=128)    # [128, 8, 1024]
    xT_r = xT[:].rearrange("(ko p) s -> p ko s", p=128)            # [128, 8, 512]
    yT_r = yT[:].rearrange("(ko p) s -> p ko s", p=128)

    with tile.TileContext(nc) as tc:
        with (
            tc.tile_pool(name="pp", bufs=1) as pp,
            tc.tile_pool(name="scratch", bufs=2) as scratch,
            tc.tile_pool(name="rows", bufs=1) as rows,
            tc.tile_pool(name="dram", bufs=1, space="DRAM") as dram,
        ):
            # ---- global constants & residual-stream tensors ----
            one1_bf = pp.tile([1, 1], BF, tag="one1")
            nc.vector.memset(one1_bf[:], 1.0)
            ones128_bf = pp.tile([128, 1], BF, tag="ones128")
            nc.vector.memset(ones128_bf[:], 1.0)
            ones1x128_f = pp.tile([1, 128], F32, tag="ones1x128")
            nc.vector.memset(ones1x128_f[:], 1.0)
            ones1x64_bf = pp.tile([1, 64], BF, tag="ones1x64")
            nc.vector.memset(ones1x64_bf[:], 1.0)
            eps_sb = pp.tile([1, 1], F32, tag="eps")
            nc.vector.memset(eps_sb[:], EPS)
            expbias_sb = pp.tile([128, 1], F32, tag="expbias")
            nc.vector.memset(expbias_sb[:], -2.5)

            cT_sb = pp.tile([128, 8], BF, tag="cT")
            nc.sync.dma_start(cT_sb[:], cT[:])
            xT_sb = pp.tile([128, 8, SL], F32, tag="xT")
            for i in range(4):
                nc.sync.dma_start(xT_sb[:, 2 * i:2 * i + 2, :], xT_r[:, 2 * i:2 * i + 2, :])
            x2T = pp.tile([128, 8, SL], F32, tag="x2T")
            mod_loc = pp.tile([128, 16], F32, tag="mod_loc")
            mod_ag = pp.tile([128, 32], F32, tag="mod_ag")
            sc1p_msa = pp.tile([128, 8], F32, tag="sc1p_msa")
            sc1p_mlp = pp.tile([128, 8], F32, tag="sc1p_mlp")
            cosd_sb = pp.tile([128, SL], BF, tag="cosd")
            nc.sync.dma_start(cosd_sb[:], cosdT[:])
            sind_sb = pp.tile([128, SL], BF, tag="sind")
            nc.sync.dma_start(sind_sb[:], sindT[:])
            pswap_sb = pp.tile([128, 128], BF, tag="pswap")
            nc.sync.dma_start(pswap_sb[:], pswap[:])
            bada_loc_sb = pp.tile([1, 4, 512], BF, tag="bada_loc")
            nc.sync.dma_start(bada_loc_sb[:], bada_loc[:])
            bada_sh_sb = pp.tile([1, 2, 512], BF, tag="bada_sh")
            nc.sync.dma_start(bada_sh_sb[:], bada_sh[:])

            def scr():
                return scratch.tile([128, SL], F32, tag="scratch", name="scr")

            # ---- LayerNorm in T-layout (stats via ones-matmul) + modulate ----
            # xn dtype comes from the destination tile (fp8 for attn, bf16 for mlp)
            def layernorm_mod(src_sb, sc1p, mod_sh, sh_col0, xn, ps_stat, ps_aux, tmp_ln):
                xbf = tmp_ln.tile([128, 8, SL], BF, tag="lnbf", name="xbf")
                nc.vector.tensor_copy(xbf[:], src_sb[:])
                x2bf = tmp_ln.tile([128, 8, SL], BF, tag="lnbf2", name="x2bf")
                nc.vector.tensor_tensor(x2bf[:], src_sb[:], src_sb[:], OP.mult)
                ps_sum = ps_stat.tile([1, SL], F32, tag="stat", name="ps_sum")
                for ko in range(8):
                    nc.tensor.matmul(ps_sum[:], ones128_bf[:], xbf[:, ko, :],
                                     start=(ko == 0), stop=(ko == 7))
                m_sb = rows.tile([1, SL], F32, tag="m", name="m_sb")
                nc.vector.tensor_scalar_mul(m_sb[:], ps_sum[:], 1.0 / D)
                ps_sq = ps_stat.tile([1, SL], F32, tag="stat", name="ps_sq")
                for ko in range(8):
                    nc.tensor.matmul(ps_sq[:], ones128_bf[:], x2bf[:, ko, :],
                                     start=(ko == 0), stop=(ko == 7))
                var_sb = rows.tile([1, SL], F32, tag="var", name="var_sb")
                nc.vector.tensor_scalar_mul(var_sb[:], ps_sq[:], 1.0 / D)
                m2_sb = rows.tile([1, SL], F32, tag="m2", name="m2_sb")
                nc.vector.tensor_tensor(m2_sb[:], m_sb[:], m_sb[:], OP.mult)
                nc.vector.tensor_tensor(var_sb[:], var_sb[:], m2_sb[:], OP.subtract)
                rstd_sb = rows.tile([1, SL], F32, tag="rstd", name="rstd_sb")
                nc.scalar.activation(rstd_sb[:], var_sb[:], AF.Sqrt,
                                     bias=eps_sb[:], scale=1.0)
                nc.vector.reciprocal(rstd_sb[:], rstd_sb[:])
                m_rep = ps_aux.tile([128, SL], F32, tag="aux", name="m_rep")
                nc.tensor.matmul(m_rep[:], ones1x128_f[:], m_sb[:], start=True, stop=True)
                rstd_rep = ps_aux.tile([128, SL], F32, tag="aux", name="rstd_rep")
                nc.tensor.matmul(rstd_rep[:], ones1x128_f[:], rstd_sb[:],
                                 start=True, stop=True)
                for ko in range(8):
                    t1 = scr()
                    nc.vector.tensor_tensor(t1[:], src_sb[:, ko, :], m_rep[:], OP.subtract)
                    t2 = scr()
                    nc.vector.tensor_tensor(t2[:], t1[:], rstd_rep[:], OP.mult)
                    nc.vector.tensor_scalar(
                        xn[:, ko, :], t2[:],
                        scalar1=sc1p[:, ko:ko + 1],
                        scalar2=mod_sh[:, sh_col0 + ko:sh_col0 + ko + 1],
                        op0=OP.mult, op1=OP.add)

            with (
                tc.tile_pool(name="q64p", bufs=1) as q64p,
            ):
                q64 = q64p.tile([64, 16, SL], F8, tag="q64")

                with (
                    tc.tile_pool(name="qscope", bufs=1) as qs,
                    tc.tile_pool(name="wada_p", bufs=2) as wada_p,
                    tc.tile_pool(name="wqkv_p", bufs=1) as wqkv_p,
                    tc.tile_pool(name="rope", bufs=3) as rope_p,
                    tc.tile_pool(name="tmp_ln", bufs=1) as tmp_ln,
                    tc.tile_pool(name="ps_mm", bufs=3, space="PSUM") as ps_mm,
                    tc.tile_pool(name="ps_stat", bufs=1, space="PSUM") as ps_stat,
                    tc.tile_pool(name="ps_aux", bufs=2, space="PSUM") as ps_aux,
                    tc.tile_pool(name="ps_avtr", bufs=2, space="PSUM") as ps_avtr,
                ):
                    # ---- adaLN modulation ----
                    # local (replicated) half: sh_msa + sc_msa = cols 0..2047
                    mod_loc_d = dram.tile([1, 2048], F32)
                    for j in range(4):
                        wada_t = wada_p.tile([128, 8, 512], BF, tag="wada", name="wada_t")
                        nc.sync.dma_start(wada_t[:], wada_loc_r[:, :, j * 512:(j + 1) * 512])
                        st = ps_stat.tile([1, 512], F32, tag="stat", name="st_mod")
                        for ko in range(8):
                            nc.tensor.matmul(st[:], cT_sb[:, ko:ko + 1], wada_t[:, ko, :],
                                             start=(ko == 0), stop=False)
                        nc.tensor.matmul(st[:], one1_bf[:], bada_loc_sb[0:1, j, :],
                                         start=False, stop=True)
                        row = rows.tile([1, 512], F32, tag="modrow", name="modrow")
                        nc.vector.tensor_copy(row[:], st[:])
                        nc.sync.dma_start(mod_loc_d[0:1, j * 512:(j + 1) * 512], row[:])
                    nc.sync.dma_start(
                        mod_loc[:],
                        mod_loc_d[:].rearrange("one (o p) -> p (one o)", p=128))
                    nc.vector.tensor_scalar_add(sc1p_msa[:], mod_loc[:, 8:16], 1.0)

                    # sharded half: this rank's 1024 of cols 2048..6143 + AllGather
                    ag_m_in = dram.tile([1, 1024], F32)
                    ag_m_out = dram.tile([G, 1024], F32)
                    for j in range(2):
                        wada_t = wada_p.tile([128, 8, 512], BF, tag="wada", name="wada_s")
                        nc.sync.dma_start(wada_t[:], wada_sh_r[:, :, j * 512:(j + 1) * 512])
                        st = ps_stat.tile([1, 512], F32, tag="stat", name="st_mods")
                        for ko in range(8):
                            nc.tensor.matmul(st[:], cT_sb[:, ko:ko + 1], wada_t[:, ko, :],
                                             start=(ko == 0), stop=False)
                        nc.tensor.matmul(st[:], one1_bf[:], bada_sh_sb[0:1, j, :],
                                         start=False, stop=True)
                        row = rows.tile([1, 512], F32, tag="modrow", name="modrow_s")
                        nc.vector.tensor_copy(row[:], st[:])
                        nc.sync.dma_start(ag_m_in[0:1, j * 512:(j + 1) * 512], row[:])
                    nc.gpsimd.collective_compute(
                        "AllGather", OP.bypass, replica_groups=RG,
                        ins=[ag_m_in.opt()], outs=[ag_m_out.opt()])
                    nc.sync.dma_start(
                        mod_ag[:],
                        ag_m_out[:].rearrange("r (o p) -> p (r o)", p=128))
                    # fold the 1/WS dequant of the fp8 out-proj into the g_msa gate
                    nc.vector.tensor_scalar_mul(mod_ag[:, 0:8], mod_ag[:, 0:8], 1.0 / WS)
                    nc.vector.tensor_scalar_add(sc1p_mlp[:], mod_ag[:, 16:24], 1.0)
                    if DEBUG:
                        nc.sync.dma_start(dbg["mod"][:, 0:16], mod_loc[:])
                        nc.sync.dma_start(dbg["mod"][:, 16:48], mod_ag[:])

                    ident = qs.tile([128, 128], BF, tag="ident")
                    make_identity(nc, ident[:])

                    xn1 = qs.tile([128, 8, SL], F8, tag="xn1")
                    layernorm_mod(xT_sb, sc1p_msa, mod_loc, 0, xn1, ps_stat, ps_aux, tmp_ln)
                    if DEBUG:
                        nc.sync.dma_start(dbg["xn1"][:], xn1[:])
                    # preload the Exp ACT table set during the (ACT-idle) QKV phase
                    junk = rows.tile([1, 1], F32, tag="junk", name="junk")
                    nc.scalar.activation(junk[:], eps_sb[:], AF.Exp)

                    # ---- QKV (transposed, fp8 DoubleRow) + RoPE + AllGathers ----
                    wqkv_sb = wqkv_p.tile([128, 8, 3 * D], F8, tag="wqkv")
                    # column blocks in consumption order: K, V, Q
                    for c0, c1 in ((1024, 2048), (2048, 3072), (0, 1024)):
                        nc.sync.dma_start(wqkv_sb[:, :, c0:c1], wqkv_r[:, :, c0:c1])
                    ag_k_in = dram.tile([D, SL], F8)
                    ag_k_out = dram.tile([G * D, SL], F8)
                    ag_v_in = dram.tile([SL, H * 65], F8)
                    ag_v_out = dram.tile([G * SL, H * 65], F8)
                    v_aug = qs.tile([128, 4, H, 65], F8, tag="v_aug")
                    nc.vector.memset(v_aug[:, :, :, 64:65], 1.0)

                    def rope_chunk(ps, out_dt):
                        raw = rope_p.tile([128, SL], BF, tag="raw", name="raw")
                        nc.vector.tensor_scalar_mul(raw[:], ps[:], 1.0 / WS)
                        swp = ps_avtr.tile([128, SL], F32, tag="avtr", name="swp")
                        nc.tensor.matmul(swp[:], pswap_sb[:], raw[:], start=True, stop=True)
                        t1 = rope_p.tile([128, SL], BF, tag="t1", name="t1")
                        nc.vector.tensor_tensor(t1[:], swp[:], sind_sb[:], OP.mult)
                        t2 = rope_p.tile([128, SL], BF, tag="t2", name="t2")
                        nc.vector.tensor_tensor(t2[:], raw[:], cosd_sb[:], OP.mult)
                        tag = "dst8" if out_dt == F8 else "dstb"
                        dst = rope_p.tile([128, SL], out_dt, tag=tag, name="dst")
                        nc.vector.tensor_tensor(dst[:], t1[:], t2[:], OP.add)
                        return dst

                    for nt in (2, 3, 4, 5, 0, 1):  # k first, then v, then q
                        for mc in range(4):
                            fc = nt * 4 + mc
                            ps = ps_mm.tile([128, SL], F32, tag="mm", name="ps_qkv")
                            for kp in range(4):
                                nc.tensor.matmul(
                                    ps[:],
                                    wqkv_sb[:, 2 * kp:2 * kp + 2, fc * 128:(fc + 1) * 128],
                                    xn1[:, 2 * kp:2 * kp + 2, :],
                                    start=(kp == 0), stop=(kp == 3), perf_mode=DR)
                            dst = rope_chunk(ps, BF if 16 <= fc < 24 else F8)
                            if 8 <= fc < 16:        # k chunk -> AG input (T-layout)
                                r0 = (fc - 8) * 128
                                nc.sync.dma_start(ag_k_in[r0:r0 + 128, :], dst[:])
                            elif fc >= 16:          # v chunk -> s-layout + ones col
                                hv = (fc - 16) * 2
                                for si in range(4):
                                    tp = ps_avtr.tile([128, 128], BF, tag="avtr", name="tp")
                                    nc.tensor.transpose(
                                        tp[:], dst[:, si * 128:(si + 1) * 128], ident[:])
                                    nc.vector.tensor_copy(v_aug[:, si, hv, 0:64],
                                                          tp[:, 0:64])
                                    nc.vector.tensor_copy(v_aug[:, si, hv + 1, 0:64],
                                                          tp[:, 64:128])
                            else:                   # q chunk stays local (per-head)
                                nc.sync.dma_start(q64[:, 2 * fc, :], dst[0:64, :])
                                nc.sync.dma_start(q64[:, 2 * fc + 1, :], dst[64:128, :])
                        if nt == 3:
                            nc.gpsimd.collective_compute(
                                "AllGather", OP.bypass, replica_groups=RG,
                                ins=[ag_k_in.opt()], outs=[ag_k_out.opt()])
                        if nt == 5:
                            nc.sync.dma_start(
                                ag_v_in[:].rearrange("(si p) (h w) -> p si h w",
                                                     p=128, h=H),
                                v_aug[:])
                            nc.gpsimd.collective_compute(
                                "AllGather", OP.bypass, replica_groups=RG,
                                ins=[ag_v_in.opt()], outs=[ag_v_out.opt()])
                    if DEBUG:
                        nc.sync.dma_start(dbg["q"][:], q64[:])
                        nc.sync.dma_start(dbg["agk"][:], ag_k_out[:])

                # ---- attention + out-projection ----
                # w1/hT pools open here (after the QKV scope frees its SBUF):
                # the first w1 half prefetches during attention.
                with (
                    tc.tile_pool(name="w1_p", bufs=1) as w1_p,
                    tc.tile_pool(name="hT_p", bufs=1) as hT_p,
                ):
                    w1a = w1_p.tile([128, 8, 2048], BF, tag="w1", name="w1a")
                    for ko in range(8):
                        nc.sync.dma_start(w1a[:, ko, :], w1_r[:, ko, 0:2048])
                    hT = hT_p.tile([128, 32, SL], BF, tag="hT")

                    with (
                        tc.tile_pool(name="ascope", bufs=1) as asc,
                        tc.tile_pool(name="kth_p", bufs=2) as kth_p,
                        tc.tile_pool(name="exph_p", bufs=2) as exph_p,
                        tc.tile_pool(name="recp", bufs=2) as recp,
                        tc.tile_pool(name="wout_p", bufs=2) as wout_p,
                        tc.tile_pool(name="ps_qk", bufs=1, space="PSUM") as ps_qk,
                        tc.tile_pool(name="ps_av", bufs=2, space="PSUM") as ps_av,
                        tc.tile_pool(name="ps_rec", bufs=2, space="PSUM") as ps_rec,
                    ):
                        v_full = asc.tile([128, 16, H, 65], F8, tag="v_full")
                        for r in range(G):
                            nc.sync.dma_start(
                                v_full[:, r * 4:(r + 1) * 4, :, :],
                                ag_v_out[r * SL:(r + 1) * SL, :].rearrange(
                                    "(si p) (h w) -> p si h w", p=128, h=H))
                        attn_sb = asc.tile([128, 8, SL], F8, tag="attn")
                        for h in range(H):
                            kT_h = kth_p.tile([64, G, SL], F8, tag="kth", name="kT_h")
                            for r in range(G):
                                nc.sync.dma_start(
                                    kT_h[:, r, :],
                                    ag_k_out[r * D + h * 64:r * D + (h + 1) * 64, :])
                            exp_h = exph_p.tile([128, 16, SL], F8, tag="exph",
                                                name="exp_h")
                            for mq in range(4):   # 4 key-chunks per QK psum tile
                                sc_ps = ps_qk.tile([128, 4, SL], F32, tag="qk",
                                                   name="sc_ps")
                                for i in range(4):
                                    m = 4 * mq + i
                                    nc.tensor.matmul(
                                        sc_ps[:, i, :],
                                        kT_h[:, m // 4,
                                             (m % 4) * 128:(m % 4) * 128 + 128],
                                        q64[:, h, :], start=True, stop=True)
                                # exp(score/8 - 2.5): constant bias keeps fp8 in
                                # range; it cancels against the denominator.
                                nc.scalar.activation(
                                    exp_h[:, 4 * mq:4 * mq + 4, :], sc_ps[:], AF.Exp,
                                    scale=1.0 / float(np.sqrt(HD)),
                                    bias=expbias_sb[:])
                            av = ps_av.tile([65, SL], F32, tag="av", name="av")
                            for mp in range(8):
                                nc.tensor.matmul(av[:],
                                                 v_full[:, 2 * mp:2 * mp + 2, h, :],
                                                 exp_h[:, 2 * mp:2 * mp + 2, :],
                                                 start=(mp == 0), stop=(mp == 7),
                                                 perf_mode=DR)
                            rec_f = rows.tile([1, SL], F32, tag="recf", name="rec_f")
                            nc.vector.reciprocal(rec_f[:], av[64:65, :])
                            rec = rows.tile([1, SL], BF, tag="rec", name="rec")
                            nc.vector.tensor_copy(rec[:], rec_f[:])
                            rec_rep = ps_rec.tile([64, SL], F32, tag="rec",
                                                  name="rec_rep")
                            nc.tensor.matmul(rec_rep[:], ones1x64_bf[:], rec[:],
                                             start=True, stop=True)
                            rec_rep_sb = recp.tile([64, SL], F32, tag="recrep",
                                                   name="rec_rep_sb")
                            nc.vector.tensor_copy(rec_rep_sb[:], rec_rep[:])
                            attn64 = recp.tile([64, SL], F8, tag="attn64",
                                               name="attn64")
                            nc.vector.tensor_tensor(attn64[:], av[0:64, :],
                                                    rec_rep_sb[:], OP.mult)
                            # shuffle into [128, headpair, SL] layout for out-proj
                            nc.sync.dma_start(
                                attn_sb[(h % 2) * 64:(h % 2) * 64 + 64, h // 2, :],
                                attn64[:])
                        if DEBUG:
                            nc.sync.dma_start(dbg["attn"][:], attn_sb[:])

                        # ---- out projection (fp8 DoubleRow) + gated residual ----
                        for dc in range(8):
                            wo_t = wout_p.tile([128, 8, 128], F8, tag="wo",
                                               name="wo_t")
                            nc.sync.dma_start(wo_t[:],
                                              wout_r[:, :, dc * 128:(dc + 1) * 128])
                            ps = ps_qk.tile([128, 4, SL], F32, tag="qk",
                                            name="ps_out")
                            for kp in range(4):
                                nc.tensor.matmul(
                                    ps[:, 0, :],
                                    wo_t[:, 2 * kp:2 * kp + 2, :],
                                    attn_sb[:, 2 * kp:2 * kp + 2, :],
                                    start=(kp == 0), stop=(kp == 3), perf_mode=DR)
                            tg = scr()
                            nc.vector.tensor_scalar_mul(tg[:], ps[:, 0, :],
                                                        mod_ag[:, dc:dc + 1])
                            nc.vector.tensor_tensor(x2T[:, dc, :], xT_sb[:, dc, :],
                                                    tg[:], OP.add)
                        if DEBUG:
                            nc.sync.dma_start(dbg["x2"][:], x2T[:])

                    # ---- LN2 + MLP1 (bf16) ----
                    with (
                        tc.tile_pool(name="m1scope", bufs=1) as m1s,
                        tc.tile_pool(name="tmp_ln2", bufs=1) as tmp_ln2,
                        tc.tile_pool(name="ps_stat2", bufs=1, space="PSUM") as ps_stat2,
                        tc.tile_pool(name="ps_aux2", bufs=2, space="PSUM") as ps_aux2,
                        tc.tile_pool(name="ps_m1", bufs=1, space="PSUM") as ps_m1,
                    ):
                        xn2 = m1s.tile([128, 8, SL], BF, tag="xn2")
                        layernorm_mod(x2T, sc1p_mlp, mod_ag, 8, xn2,
                                      ps_stat2, ps_aux2, tmp_ln2)
                        w1b = None
                        for mq in range(8):
                            if mq == 4:
                                w1b = w1_p.tile([128, 8, 2048], BF, tag="w1",
                                                name="w1b")
                                for ko in range(8):
                                    nc.sync.dma_start(w1b[:, ko, :],
                                                      w1_r[:, ko, 2048:4096])
                            w1x, coff = (w1a, 0) if mq < 4 else (w1b, 2048)
                            ps = ps_m1.tile([128, 4, SL], F32, tag="m1", name="ps_m1")
                            for i in range(4):
                                mt = 4 * mq + i
                                cs = mt * 128 - coff
                                for ko in range(8):
                                    nc.tensor.matmul(
                                        ps[:, i, :], w1x[:, ko, cs:cs + 128],
                                        xn2[:, ko, :], start=(ko == 0), stop=(ko == 7))
                            nc.scalar.activation(hT[:, 4 * mq:4 * mq + 4, :], ps[:],
                                                 AF.Gelu_apprx_tanh)

                    # ---- MLP2 (bf16) + gated residual + output ----
                    with (
                        tc.tile_pool(name="w2_p", bufs=3) as w2_p,
                        tc.tile_pool(name="ps_m2", bufs=2, space="PSUM") as ps_m2,
                    ):
                        for dc in range(8):
                            ps = ps_m2.tile([128, SL], F32, tag="m2", name="ps_m2")
                            for kq in range(2):
                                w2_t = w2_p.tile([128, 16, 128], BF, tag="w2",
                                                 name="w2_t")
                                nc.sync.dma_start(
                                    w2_t[:],
                                    w2_r[:, kq * 16:(kq + 1) * 16,
                                         dc * 128:(dc + 1) * 128])
                                for kk in range(16):
                                    kt = kq * 16 + kk
                                    nc.tensor.matmul(ps[:], w2_t[:, kk, :],
                                                     hT[:, kt, :],
                                                     start=(kt == 0), stop=(kt == 31))
                            tg = scr()
                            nc.vector.tensor_scalar_mul(tg[:], ps[:],
                                                        mod_ag[:, 24 + dc:25 + dc])
                            yrow = scr()
                            nc.vector.tensor_tensor(yrow[:], x2T[:, dc, :], tg[:],
                                                    OP.add)
                            nc.sync.dma_start(yT_r[:, dc, :], yrow[:])

    return nc


_NC_CACHE = None


def _prep_in_maps(inputs):
    x = np.asarray(inputs["x"], dtype=np.float32)
    c = np.asarray(inputs["c"], dtype=np.float32)
    cos = np.asarray(inputs["cos"], dtype=np.float32)
    sin = np.asarray(inputs["sin"], dtype=np.float32)

    def b16(a):
        return np.ascontiguousarray(a).astype(ml_dtypes.bfloat16)

    def f8(a, scale=1.0):
        return (np.ascontiguousarray(a) * scale).astype(ml_dtypes.float8_e4m3)

    wqkv_8 = f8(inputs["W_qkv"], WS)
    wout_8 = f8(inputs["W_out"], WS)
    w1_b = b16(inputs["W1"])
    w2_b = b16(inputs["W2"])
    wada_full = b16(inputs["W_ada"])
    bada_full = b16(np.asarray(inputs["b_ada"], dtype=np.float32).reshape(1, 12, 512))

    jj = np.arange(128) % 64
    pair = jj // 2
    sign = np.where(jj % 2 == 0, -1.0, 1.0).astype(np.float32)
    pswap_m = np.zeros((128, 128), np.float32)
    pswap_m[np.arange(128) ^ 1, np.arange(128)] = 1.0
    pswap_m = b16(pswap_m)

    in_maps = []
    for g in range(8):
        b, r = g // G, g % G
        rows = slice(r * SL, (r + 1) * SL)
        cl = cos[rows, 0:HD // 2]     # [512, 32]
        sl = sin[rows, 0:HD // 2]
        cosdT_m = b16(np.ascontiguousarray(cl.T[pair]))             # [128, 512]
        sindT_m = b16(np.ascontiguousarray(sl.T[pair] * sign[:, None]))
        in_maps.append({
            "xT": np.ascontiguousarray(x[b, rows, :].T),
            "cT": b16(c[b].reshape(8, 128).T),
            "bada_loc": np.ascontiguousarray(bada_full[:, 0:4, :]),
            "bada_sh": np.ascontiguousarray(bada_full[:, 4 + 2 * r:6 + 2 * r, :]),
            "cosdT": cosdT_m,
            "sindT": sindT_m,
            "pswap": pswap_m,
            "wqkv": wqkv_8, "wout": wout_8, "w1": w1_b, "w2": w2_b,
            "wada_loc": np.ascontiguousarray(wada_full[:, 0:2048]),
            "wada_sh": np.ascontiguousarray(
                wada_full[:, 2048 + 1024 * r:2048 + 1024 * (r + 1)]),
        })
    return in_maps


LAST_RESULT = None


def kernel(**inputs) -> np.ndarray:
    global _NC_CACHE, LAST_RESULT
    if _NC_CACHE is None:
        _NC_CACHE = build()
    nc = _NC_CACHE
    in_maps = _prep_in_maps(inputs)
    res = run_bass_kernel_spmd(nc, in_maps, core_ids=list(range(8)))
    LAST_RESULT = res
    y = np.empty((B, S, D), np.float32)
    for g in range(8):
        b, r = g // G, g % G
        y[b, r * SL:(r + 1) * SL, :] = res.results[g]["yT"].T
    return y


# revision 23
# speedup vs baseline: 1.3016x; 1.2705x over previous
"""DDiT block (adaLN transformer block) on 8 Trainium2 NeuronCores.

Sharding: sequence-parallel everywhere + per-batch K/V AllGather (Ulysses-style).
  - 8 cores = 2 batch groups x 4 sequence ranks. Core g handles batch b=g//4,
    rows [r*512, (r+1)*512) with r=g%4.
  - adaLN mod: the msa sh/sc half (cols 0..2047) is computed replicated on
    every core (no collective on the critical path); the rest (g_msa + mlp
    mods, cols 2048..6143) is computed 1/4-sharded and AllGathered (the AG
    rides out the one-time CC bootstrap barrier).
  - QKV projection in fp8 DoubleRow (weights pre-scaled x16 on host), RoPE in
    bf16, K^T / V AllGathers carry fp8 payloads, attention scores in fp8,
    softmax exp on ACT with fused bias (no max-subtraction; constant folded
    into exp cancels in the denominator), AV + out-proj in fp8 DoubleRow.
  - MLP stays bf16 (the adaLN gates are ~0.64 rms so the MLP branch carries
    ~16% of the output energy; fp8 there costs ~1.5e-2 rel err).

Layout: activations are kept transposed ("T-layout", feature-on-partition,
sequence-on-free). The attention AV product consumes softmax(scores)
transposed; the softmax denominator comes from an appended ones-column in V.
"""
import os
import sys

sys.path.insert(0, "/opt/trn_rl_repo")

import numpy as np
import ml_dtypes

import concourse.bass as bass
import concourse.mybir as mybir
import concourse.tile as tile
from concourse.bass_utils import run_bass_kernel_spmd
from concourse.masks import make_identity
from concourse.vector_clock import ScopedClock
import bass_rust

BF = mybir.dt.bfloat16
F32 = mybir.dt.float32
F8 = mybir.dt.float8e4
AF = mybir.ActivationFunctionType
OP = mybir.AluOpType
DR = mybir.MatmulPerfMode.DoubleRow

B, S, D, H, HD, COND, MLP_H = 2, 2048, 1024, 16, 64, 1024, 4096
G = 4              # ranks per batch group
SL = S // G        # 512 local rows
EPS = 1e-6
WS = 16.0          # host-side prescale on fp8 weights (wqkv, wout)
RG = [[0, 1, 2, 3], [4, 5, 6, 7]]
DEBUG = bool(int(os.environ.get("KBENCH_DEBUG", "0")))


def _patched_drain_and_barrier(self, tick_clock, wait_clock):
    # This build's rust layer allows only one sem wait per instruction; stock
    # TileContext crams every final wait onto a single Drain, which walrus
    # rejects ("Too many sync wait commands"). Spread them over nops.
    nc = self.nc
    probe = nc.sync.nop(nofuse=True)
    wait_clock.add_sem_waits(probe.ins, ScopedClock({None: tick_clock.global_clock}))
    waits = list(probe.ins.sync_info.on_wait)
    probe.ins.sync_info.on_wait = waits[:1]
    for w in waits[1:]:
        n2 = nc.sync.nop(nofuse=True)
        n2.ins.sync_info = bass_rust.SyncInfo(on_wait=[w], on_update=[])
    nc.sync.drain()
    nc.all_engine_barrier()
    assert self.sems is not None
    popped = nc._tile_sem_poison_stack.pop()
    assert popped is self._sem_poison
    nc.clear_and_free_semaphores(list(self.sems.allocated().values()))
    nc.all_engine_barrier()


tile.TileContext._drain_and_barrier = _patched_drain_and_barrier

_orig_to_json_bytes = bass.Bass.to_json_bytes


def _to_json_bytes_split_waits(self):
    """This walrus build accepts at most one sem wait per instruction, but
    Tile's sem assignment attaches several. Spill excess waits onto freshly
    inserted EventSemaphore instructions on the same engine, just before the
    over-committed instruction (per-engine program order preserved)."""
    import json as _json
    d = _json.loads(_orig_to_json_bytes(self))
    ctr = 0
    for f in d.get("functions", []):
        for blk in f.get("blocks", []):
            out = []
            for inst in blk.get("instructions", []):
                si = inst.get("sync_info")
                waits = (si or {}).get("on_wait") or []
                if len(waits) > 1:
                    for w in waits[:-1]:
                        ctr += 1
                        ev = {
                            "engine": inst.get("engine"),
                            "ins": [],
                            "name": f"evsplit_{ctr}",
                            "opcode": "EventSemaphore",
                            "outs": [],
                            "sync_info": {"on_update": [], "on_wait": [w]},
                        }
                        if "debug" in inst:
                            ev["debug"] = inst["debug"]
                        out.append(ev)
                    si["on_wait"] = waits[-1:]
                out.append(inst)
            blk["instructions"] = out
    return _json.dumps(d).encode()


bass.Bass.to_json_bytes = _to_json_bytes_split_waits


def build():
    nc = bass.Bass(num_devices=8)

    # ---- I/O ----
    xT = nc.dram_tensor("xT", [D, SL], F32, kind="ExternalInput")
    cT = nc.dram_tensor("cT", [128, COND // 128], BF, kind="ExternalInput")
    bada_loc = nc.dram_tensor("bada_loc", [1, 4, 512], BF, kind="ExternalInput")
    bada_sh = nc.dram_tensor("bada_sh", [1, 2, 512], BF, kind="ExternalInput")
    cosdT = nc.dram_tensor("cosdT", [128, SL], BF, kind="ExternalInput")
    sindT = nc.dram_tensor("sindT", [128, SL], BF, kind="ExternalInput")
    pswap = nc.dram_tensor("pswap", [128, 128], BF, kind="ExternalInput")
    wqkv = nc.dram_tensor("wqkv", [D, 3 * D], F8, kind="ExternalInput")
    wout = nc.dram_tensor("wout", [8, 128, 8 * 128], F8, kind="ExternalInput")
    w1 = nc.dram_tensor("w1", [D, MLP_H], BF, kind="ExternalInput")
    # w2/wout are host-permuted dc-major so per-dc DMAs read contiguous lines
    w2 = nc.dram_tensor("w2", [8, 128, 32 * 128], BF, kind="ExternalInput")
    wada_loc = nc.dram_tensor("wada_loc", [COND, 2048], BF, kind="ExternalInput")
    wada_sh = nc.dram_tensor("wada_sh", [COND, 1024], BF, kind="ExternalInput")
    yT = nc.dram_tensor("yT", [D, SL], F32, kind="ExternalOutput")

    dbg = {}
    if DEBUG:
        dbg["mod"] = nc.dram_tensor("dbg_mod", [128, 48], F32, kind="ExternalOutput")
        dbg["xn1"] = nc.dram_tensor("dbg_xn1", [128, 8, SL], F8, kind="ExternalOutput")
        dbg["q"] = nc.dram_tensor("dbg_q", [64, 16, SL], F8, kind="ExternalOutput")
        dbg["attn"] = nc.dram_tensor("dbg_attn", [128, 8, SL], F8, kind="ExternalOutput")
        dbg["x2"] = nc.dram_tensor("dbg_x2", [128, 8, SL], F32, kind="ExternalOutput")
        dbg["agk"] = nc.dram_tensor("dbg_agk", [G * D, SL], F8, kind="ExternalOutput")

    wqkv_r = wqkv[:].rearrange("(ko p) f -> p ko f", p=128)        # [128, 8, 3072]
    wout_r = wout[:].rearrange("dc p (ko m) -> dc p ko m", ko=8)   # [8, 128, 8, 128]
    w1_r = w1[:].rearrange("(ko p) f -> p ko f", p=128)            # [128, 8, 4096]
    w2_r = w2[:].rearrange("dc p (kt m) -> dc p kt m", kt=32)      # [8, 128, 32, 128]
    wada_loc_r = wada_loc[:].rearrange("(ko p) f -> p ko f", p=128)  # [128, 8, 2048]
    wada_sh_r = wada_sh[:].rearrange("(ko p) f -> p ko f", p=128)    # [128, 8, 1024]
    xT_r = xT[:].rearrange("(ko p) s -> p ko s", p=128)            # [128, 8, 512]
    yT_r = yT[:].rearrange("(ko p) s -> p ko s", p=128)

    with tile.TileContext(nc) as tc:
        with (
            tc.tile_pool(name="pp", bufs=1) as pp,
            tc.tile_pool(name="scratch", bufs=2) as scratch,
            tc.tile_pool(name="rows", bufs=1) as rows,
            tc.tile_pool(name="dram", bufs=1, space="DRAM") as dram,
        ):
            # ---- global constants & residual-stream tensors ----
            one1_bf = pp.tile([1, 1], BF, tag="one1")
            nc.vector.memset(one1_bf[:], 1.0)
            ones128_bf = pp.tile([128, 1], BF, tag="ones128")
            nc.vector.memset(ones128_bf[:], 1.0)
            ones1x128_f = pp.tile([1, 128], F32, tag="ones1x128")
            nc.vector.memset(ones1x128_f[:], 1.0)
            ones1x64_bf = pp.tile([1, 64], BF, tag="ones1x64")
            nc.vector.memset(ones1x64_bf[:], 1.0)
            eps_sb = pp.tile([1, 1], F32, tag="eps")
            nc.vector.memset(eps_sb[:], EPS)
            expbias_sb = pp.tile([128, 1], F32, tag="expbias")
            nc.vector.memset(expbias_sb[:], -2.5)

            cT_sb = pp.tile([128, 8], BF, tag="cT")
            nc.sync.dma_start(cT_sb[:], cT[:])
            xT_sb = pp.tile([128, 8, SL], F32, tag="xT")
            for i in range(4):
                nc.sync.dma_start(xT_sb[:, 2 * i:2 * i + 2, :], xT_r[:, 2 * i:2 * i + 2, :])
            x2T = pp.tile([128, 8, SL], F32, tag="x2T")
            mod_loc = pp.tile([128, 16], F32, tag="mod_loc")
            mod_ag = pp.tile([128, 32], F32, tag="mod_ag")
            sc1p_msa = pp.tile([128, 8], F32, tag="sc1p_msa")
            sc1p_mlp = pp.tile([128, 8], F32, tag="sc1p_mlp")
            cosd_sb = pp.tile([128, SL], BF, tag="cosd")
            nc.sync.dma_start(cosd_sb[:], cosdT[:])
            sind_sb = pp.tile([128, SL], BF, tag="sind")
            nc.sync.dma_start(sind_sb[:], sindT[:])
            pswap_sb = pp.tile([128, 128], BF, tag="pswap")
            nc.sync.dma_start(pswap_sb[:], pswap[:])
            bada_loc_sb = pp.tile([1, 4, 512], BF, tag="bada_loc")
            nc.sync.dma_start(bada_loc_sb[:], bada_loc[:])
            bada_sh_sb = pp.tile([1, 2, 512], BF, tag="bada_sh")
            nc.sync.dma_start(bada_sh_sb[:], bada_sh[:])

            def scr():
                return scratch.tile([128, SL], F32, tag="scratch", name="scr")

            # ---- LayerNorm in T-layout (stats via ones-matmul) + modulate ----
            # xn dtype comes from the destination tile (fp8 for attn, bf16 for mlp)
            def layernorm_mod(src_sb, sc1p, mod_sh, sh_col0, xn, ps_stat, ps_aux, tmp_ln):
                xbf = tmp_ln.tile([128, 8, SL], BF, tag="lnbf", name="xbf")
                nc.vector.tensor_copy(xbf[:], src_sb[:])
                x2bf = tmp_ln.tile([128, 8, SL], BF, tag="lnbf2", name="x2bf")
                nc.vector.tensor_tensor(x2bf[:], xbf[:], xbf[:], OP.mult)
                ps_sum = ps_stat.tile([1, SL], F32, tag="stat", name="ps_sum")
                for ko in range(8):
                    nc.tensor.matmul(ps_sum[:], ones128_bf[:], xbf[:, ko, :],
                                     start=(ko == 0), stop=(ko == 7))
                m_sb = rows.tile([1, SL], F32, tag="m", name="m_sb")
                nc.vector.tensor_scalar_mul(m_sb[:], ps_sum[:], 1.0 / D)
                ps_sq = ps_stat.tile([1, SL], F32, tag="stat", name="ps_sq")
                for ko in range(8):
                    nc.tensor.matmul(ps_sq[:], ones128_bf[:], x2bf[:, ko, :],
                                     start=(ko == 0), stop=(ko == 7))
                var_sb = rows.tile([1, SL], F32, tag="var", name="var_sb")
                nc.vector.tensor_scalar_mul(var_sb[:], ps_sq[:], 1.0 / D)
                m2_sb = rows.tile([1, SL], F32, tag="m2", name="m2_sb")
                nc.vector.tensor_tensor(m2_sb[:], m_sb[:], m_sb[:], OP.mult)
                nc.vector.tensor_tensor(var_sb[:], var_sb[:], m2_sb[:], OP.subtract)
                rstd_sb = rows.tile([1, SL], F32, tag="rstd", name="rstd_sb")
                nc.scalar.activation(rstd_sb[:], var_sb[:], AF.Sqrt,
                                     bias=eps_sb[:], scale=1.0)
                nc.vector.reciprocal(rstd_sb[:], rstd_sb[:])
                m_rep = ps_aux.tile([128, SL], F32, tag="aux", name="m_rep")
                nc.tensor.matmul(m_rep[:], ones1x128_f[:], m_sb[:], start=True, stop=True)
                rstd_rep = ps_aux.tile([128, SL], F32, tag="aux", name="rstd_rep")
                nc.tensor.matmul(rstd_rep[:], ones1x128_f[:], rstd_sb[:],
                                 start=True, stop=True)
                for ko in range(8):
                    t1 = scr()
                    nc.vector.tensor_tensor(t1[:], src_sb[:, ko, :], m_rep[:], OP.subtract)
                    t2 = scr()
                    nc.vector.tensor_tensor(t2[:], t1[:], rstd_rep[:], OP.mult)
                    nc.vector.tensor_scalar(
                        xn[:, ko, :], t2[:],
                        scalar1=sc1p[:, ko:ko + 1],
                        scalar2=mod_sh[:, sh_col0 + ko:sh_col0 + ko + 1],
                        op0=OP.mult, op1=OP.add)

            with (
                tc.tile_pool(name="q64p", bufs=1) as q64p,
            ):
                q64 = q64p.tile([64, 16, SL], F8, tag="q64")

                with (
                    tc.tile_pool(name="qscope", bufs=1) as qs,
                    tc.tile_pool(name="wada_p", bufs=2) as wada_p,
                    tc.tile_pool(name="wqkv_p", bufs=1) as wqkv_p,
                    tc.tile_pool(name="rope", bufs=3) as rope_p,
                    tc.tile_pool(name="tmp_ln", bufs=1) as tmp_ln,
                    tc.tile_pool(name="ps_mm", bufs=2, space="PSUM") as ps_mm,
                    tc.tile_pool(name="ps_stat", bufs=2, space="PSUM") as ps_stat,
                    tc.tile_pool(name="ps_aux", bufs=2, space="PSUM") as ps_aux,
                    tc.tile_pool(name="ps_avtr", bufs=2, space="PSUM") as ps_avtr,
                ):
                    # ---- adaLN modulation ----
                    # local (replicated) half: sh_msa + sc_msa = cols 0..2047
                    mod_loc_d = dram.tile([1, 2048], F32)
                    for j in range(4):
                        wada_t = wada_p.tile([128, 8, 512], BF, tag="wada", name="wada_t")
                        nc.sync.dma_start(wada_t[:], wada_loc_r[:, :, j * 512:(j + 1) * 512])
                        st = ps_stat.tile([1, 512], F32, tag="stat", name="st_mod")
                        for ko in range(8):
                            nc.tensor.matmul(st[:], cT_sb[:, ko:ko + 1], wada_t[:, ko, :],
                                             start=(ko == 0), stop=False)
                        nc.tensor.matmul(st[:], one1_bf[:], bada_loc_sb[0:1, j, :],
                                         start=False, stop=True)
                        row = rows.tile([1, 512], F32, tag="modrow", name="modrow")
                        nc.vector.tensor_copy(row[:], st[:])
                        nc.sync.dma_start(mod_loc_d[0:1, j * 512:(j + 1) * 512], row[:])
                    nc.sync.dma_start(
                        mod_loc[:],
                        mod_loc_d[:].rearrange("one (o p) -> p (one o)", p=128))
                    nc.vector.tensor_scalar_add(sc1p_msa[:], mod_loc[:, 8:16], 1.0)

                    # sharded half: this rank's 1024 of cols 2048..6143 + AllGather
                    ag_m_in = dram.tile([1, 1024], F32)
                    ag_m_out = dram.tile([G, 1024], F32)
                    for j in range(2):
                        wada_t = wada_p.tile([128, 8, 512], BF, tag="wada", name="wada_s")
                        nc.sync.dma_start(wada_t[:], wada_sh_r[:, :, j * 512:(j + 1) * 512])
                        st = ps_stat.tile([1, 512], F32, tag="stat", name="st_mods")
                        for ko in range(8):
                            nc.tensor.matmul(st[:], cT_sb[:, ko:ko + 1], wada_t[:, ko, :],
                                             start=(ko == 0), stop=False)
                        nc.tensor.matmul(st[:], one1_bf[:], bada_sh_sb[0:1, j, :],
                                         start=False, stop=True)
                        row = rows.tile([1, 512], F32, tag="modrow", name="modrow_s")
                        nc.vector.tensor_copy(row[:], st[:])
                        nc.sync.dma_start(ag_m_in[0:1, j * 512:(j + 1) * 512], row[:])
                    nc.gpsimd.collective_compute(
                        "AllGather", OP.bypass, replica_groups=RG,
                        ins=[ag_m_in.opt()], outs=[ag_m_out.opt()])
                    nc.sync.dma_start(
                        mod_ag[:],
                        ag_m_out[:].rearrange("r (o p) -> p (r o)", p=128))
                    # fold the 1/WS dequant of the fp8 out-proj into the g_msa gate
                    nc.vector.tensor_scalar_mul(mod_ag[:, 0:8], mod_ag[:, 0:8], 1.0 / WS)
                    nc.vector.tensor_scalar_add(sc1p_mlp[:], mod_ag[:, 16:24], 1.0)
                    if DEBUG:
                        nc.sync.dma_start(dbg["mod"][:, 0:16], mod_loc[:])
                        nc.sync.dma_start(dbg["mod"][:, 16:48], mod_ag[:])

                    ident = qs.tile([128, 128], BF, tag="ident")
                    make_identity(nc, ident[:])

                    xn1 = qs.tile([128, 8, SL], F8, tag="xn1")
                    layernorm_mod(xT_sb, sc1p_msa, mod_loc, 0, xn1, ps_stat, ps_aux, tmp_ln)
                    if DEBUG:
                        nc.sync.dma_start(dbg["xn1"][:], xn1[:])
                    # preload the Exp ACT table set during the (ACT-idle) QKV phase
                    junk = rows.tile([1, 1], F32, tag="junk", name="junk")
                    nc.scalar.activation(junk[:], eps_sb[:], AF.Exp)

                    # ---- QKV (transposed, fp8 DoubleRow) + RoPE + AllGathers ----
                    wqkv_sb = wqkv_p.tile([128, 8, 3 * D], F8, tag="wqkv")
                    # column blocks in consumption order: K, V, Q
                    for c0, c1 in ((1024, 2048), (2048, 3072), (0, 1024)):
                        nc.sync.dma_start(wqkv_sb[:, :, c0:c1], wqkv_r[:, :, c0:c1])
                    ag_k_in = dram.tile([D, SL], F8)
                    ag_k_out = dram.tile([G * D, SL], F8)
                    ag_v_in = dram.tile([SL, H * 65], F8)
                    ag_v_out = dram.tile([G * SL, H * 65], F8)
                    v_aug = qs.tile([128, 4, H, 65], F8, tag="v_aug")
                    nc.vector.memset(v_aug[:, :, :, 64:65], 1.0)

                    def rope_chunk(ps, out_dt):
                        raw = rope_p.tile([128, SL], BF, tag="raw", name="raw")
                        # PSUM->SBUF dequant copy on the (idle) ACT engine
                        nc.scalar.activation(raw[:], ps[:], AF.Copy, scale=1.0 / WS)
                        swp = ps_avtr.tile([128, SL], F32, tag="avtr", name="swp")
                        nc.tensor.matmul(swp[:], pswap_sb[:], raw[:], start=True, stop=True)
                        t1 = rope_p.tile([128, SL], BF, tag="t1", name="t1")
                        nc.vector.tensor_tensor(t1[:], swp[:], sind_sb[:], OP.mult)
                        t2 = rope_p.tile([128, SL], BF, tag="t2", name="t2")
                        nc.vector.tensor_tensor(t2[:], raw[:], cosd_sb[:], OP.mult)
                        tag = "dst8" if out_dt == F8 else "dstb"
                        dst = rope_p.tile([128, SL], out_dt, tag=tag, name="dst")
                        nc.vector.tensor_tensor(dst[:], t1[:], t2[:], OP.add)
                        return dst

                    for nt in (2, 3, 4, 5, 0, 1):  # k first, then v, then q
                        for mc in range(4):
                            fc = nt * 4 + mc
                            ps = ps_mm.tile([128, SL], F32, tag="mm", name="ps_qkv")
                            for kp in range(4):
                                nc.tensor.matmul(
                                    ps[:],
                                    wqkv_sb[:, 2 * kp:2 * kp + 2, fc * 128:(fc + 1) * 128],
                                    xn1[:, 2 * kp:2 * kp + 2, :],
                                    start=(kp == 0), stop=(kp == 3), perf_mode=DR)
                            dst = rope_chunk(ps, BF if 16 <= fc < 24 else F8)
                            if 8 <= fc < 16:        # k chunk -> AG input (T-layout)
                                r0 = (fc - 8) * 128
                                nc.sync.dma_start(ag_k_in[r0:r0 + 128, :], dst[:])
                            elif fc >= 16:          # v chunk -> s-layout + ones col
                                hv = (fc - 16) * 2
                                for si in range(4):
                                    tp = ps_avtr.tile([128, 128], BF, tag="avtr", name="tp")
                                    nc.tensor.transpose(
                                        tp[:], dst[:, si * 128:(si + 1) * 128], ident[:])
                                    nc.vector.tensor_copy(v_aug[:, si, hv, 0:64],
                                                          tp[:, 0:64])
                                    nc.vector.tensor_copy(v_aug[:, si, hv + 1, 0:64],
                                                          tp[:, 64:128])
                            else:                   # q chunk stays local (per-head)
                                nc.sync.dma_start(q64[:, 2 * fc, :], dst[0:64, :])
                                nc.sync.dma_start(q64[:, 2 * fc + 1, :], dst[64:128, :])
                        if nt == 3:
                            nc.gpsimd.collective_compute(
                                "AllGather", OP.bypass, replica_groups=RG,
                                ins=[ag_k_in.opt()], outs=[ag_k_out.opt()])
                        if nt == 5:
                            nc.sync.dma_start(
                                ag_v_in[:].rearrange("(si p) (h w) -> p si h w",
                                                     p=128, h=H),
                                v_aug[:])
                            nc.gpsimd.collective_compute(
                                "AllGather", OP.bypass, replica_groups=RG,
                                ins=[ag_v_in.opt()], outs=[ag_v_out.opt()])
                    if DEBUG:
                        nc.sync.dma_start(dbg["q"][:], q64[:])
                        nc.sync.dma_start(dbg["agk"][:], ag_k_out[:])

                # ---- attention + out-projection ----
                # w1/hT pools open here (after the QKV scope frees its SBUF):
                # the first w1 half prefetches during attention.
                with (
                    tc.tile_pool(name="w1_p", bufs=1) as w1_p,
                    tc.tile_pool(name="hT_p", bufs=1) as hT_p,
                ):
                    w1a = w1_p.tile([128, 8, 2048], BF, tag="w1", name="w1a")
                    for ko in range(8):
                        nc.sync.dma_start(w1a[:, ko, :], w1_r[:, ko, 0:2048])
                    hT = hT_p.tile([128, 32, SL], BF, tag="hT")

                    with (
                        tc.tile_pool(name="ascope", bufs=1) as asc,
                        tc.tile_pool(name="kth_p", bufs=2) as kth_p,
                        tc.tile_pool(name="exph_p", bufs=2) as exph_p,
                        tc.tile_pool(name="recp", bufs=2) as recp,
                        tc.tile_pool(name="wout_p", bufs=2) as wout_p,
                        tc.tile_pool(name="ps_qk", bufs=2, space="PSUM") as ps_qk,
                        tc.tile_pool(name="ps_av", bufs=2, space="PSUM") as ps_av,
                        tc.tile_pool(name="ps_rec", bufs=2, space="PSUM") as ps_rec,
                    ):
                        v_full = asc.tile([128, 16, H, 65], F8, tag="v_full")
                        for r in range(G):
                            nc.sync.dma_start(
                                v_full[:, r * 4:(r + 1) * 4, :, :],
                                ag_v_out[r * SL:(r + 1) * SL, :].rearrange(
                                    "(si p) (h w) -> p si h w", p=128, h=H))
                        attn_sb = asc.tile([128, 8, SL], F8, tag="attn")
                        for h in range(H):
                            kT_h = kth_p.tile([64, G, SL], F8, tag="kth", name="kT_h")
                            for r in range(G):
                                nc.sync.dma_start(
                                    kT_h[:, r, :],
                                    ag_k_out[r * D + h * 64:r * D + (h + 1) * 64, :])
                            exp_h = exph_p.tile([128, 16, SL], F8, tag="exph",
                                                name="exp_h")
                            for mq in range(8):   # 2 key-chunks per QK psum tile
                                sc_ps = ps_qk.tile([128, 2, SL], F32, tag="qk",
                                                   name="sc_ps")
                                for i in range(2):
                                    m = 2 * mq + i
                                    nc.tensor.matmul(
                                        sc_ps[:, i, :],
                                        kT_h[:, m // 4,
                                             (m % 4) * 128:(m % 4) * 128 + 128],
                                        q64[:, h, :], start=True, stop=True)
                                # exp(score/8 - 2.5): constant bias keeps fp8 in
                                # range; it cancels against the denominator.
                                nc.scalar.activation(
                                    exp_h[:, 2 * mq:2 * mq + 2, :], sc_ps[:], AF.Exp,
                                    scale=1.0 / float(np.sqrt(HD)),
                                    bias=expbias_sb[:])
                            av = ps_av.tile([65, SL], F32, tag="av", name="av")
                            for mp in range(8):
                                nc.tensor.matmul(av[:],
                                                 v_full[:, 2 * mp:2 * mp + 2, h, :],
                                                 exp_h[:, 2 * mp:2 * mp + 2, :],
                                                 start=(mp == 0), stop=(mp == 7),
                                                 perf_mode=DR)
                            rec_f = rows.tile([1, SL], F32, tag="recf", name="rec_f")
                            nc.vector.reciprocal(rec_f[:], av[64:65, :])
                            rec = rows.tile([1, SL], BF, tag="rec", name="rec")
                            nc.vector.tensor_copy(rec[:], rec_f[:])
                            rec_rep = ps_rec.tile([64, SL], F32, tag="rec",
                                                  name="rec_rep")
                            nc.tensor.matmul(rec_rep[:], ones1x64_bf[:], rec[:],
                                             start=True, stop=True)
                            rec_rep_sb = recp.tile([64, SL], F32, tag="recrep",
                                                   name="rec_rep_sb")
                            nc.vector.tensor_copy(rec_rep_sb[:], rec_rep[:])
                            attn64 = recp.tile([64, SL], F8, tag="attn64",
                                               name="attn64")
                            nc.vector.tensor_tensor(attn64[:], av[0:64, :],
                                                    rec_rep_sb[:], OP.mult)
                            # shuffle into [128, headpair, SL] layout for out-proj
                            nc.sync.dma_start(
                                attn_sb[(h % 2) * 64:(h % 2) * 64 + 64, h // 2, :],
                                attn64[:])
                        if DEBUG:
                            nc.sync.dma_start(dbg["attn"][:], attn_sb[:])

                        # ---- out projection (fp8 DoubleRow) + gated residual ----
                        for dc in range(8):
                            wo_t = wout_p.tile([128, 8, 128], F8, tag="wo",
                                               name="wo_t")
                            nc.sync.dma_start(wo_t[:], wout_r[dc])
                            ps = ps_qk.tile([128, 2, SL], F32, tag="qk",
                                            name="ps_out")
                            for kp in range(4):
                                nc.tensor.matmul(
                                    ps[:, 0, :],
                                    wo_t[:, 2 * kp:2 * kp + 2, :],
                                    attn_sb[:, 2 * kp:2 * kp + 2, :],
                                    start=(kp == 0), stop=(kp == 3), perf_mode=DR)
                            tg = scr()
                            nc.vector.tensor_scalar_mul(tg[:], ps[:, 0, :],
                                                        mod_ag[:, dc:dc + 1])
                            nc.vector.tensor_tensor(x2T[:, dc, :], xT_sb[:, dc, :],
                                                    tg[:], OP.add)
                        if DEBUG:
                            nc.sync.dma_start(dbg["x2"][:], x2T[:])

                    # ---- LN2 + MLP1 (bf16) ----
                    with tc.tile_pool(name="m1scope", bufs=1) as m1s:
                        xn2 = m1s.tile([128, 8, SL], BF, tag="xn2")
                        with (
                            tc.tile_pool(name="tmp_ln2", bufs=1) as tmp_ln2,
                            tc.tile_pool(name="ps_stat2", bufs=2, space="PSUM")
                            as ps_stat2,
                            tc.tile_pool(name="ps_aux2", bufs=2, space="PSUM")
                            as ps_aux2,
                        ):
                            layernorm_mod(x2T, sc1p_mlp, mod_ag, 8, xn2,
                                          ps_stat2, ps_aux2, tmp_ln2)
                        with tc.tile_pool(name="ps_m1", bufs=2, space="PSUM") as ps_m1:
                            w1b = None
                            for mq in range(8):
                                if mq == 4:
                                    w1b = w1_p.tile([128, 8, 2048], BF, tag="w1",
                                                    name="w1b")
                                    for ko in range(8):
                                        nc.sync.dma_start(w1b[:, ko, :],
                                                          w1_r[:, ko, 2048:4096])
                                w1x, coff = (w1a, 0) if mq < 4 else (w1b, 2048)
                                ps = ps_m1.tile([128, 4, SL], F32, tag="m1",
                                                name="ps_m1")
                                for i in range(4):
                                    mt = 4 * mq + i
                                    cs = mt * 128 - coff
                                    for ko in range(8):
                                        nc.tensor.matmul(
                                            ps[:, i, :], w1x[:, ko, cs:cs + 128],
                                            xn2[:, ko, :],
                                            start=(ko == 0), stop=(ko == 7))
                                nc.scalar.activation(hT[:, 4 * mq:4 * mq + 4, :],
                                                     ps[:], AF.Gelu_apprx_tanh)

                    # ---- MLP2 (bf16) + gated residual + output ----
                    with (
                        tc.tile_pool(name="w2_p", bufs=2) as w2_p,
                        tc.tile_pool(name="ps_m2", bufs=2, space="PSUM") as ps_m2,
                    ):
                        for dc in range(8):
                            w2_t = w2_p.tile([128, 32, 128], BF, tag="w2",
                                             name="w2_t")
                            nc.sync.dma_start(w2_t[:], w2_r[dc])
                            ps = ps_m2.tile([128, SL], F32, tag="m2", name="ps_m2")
                            for kt in range(32):
                                nc.tensor.matmul(ps[:], w2_t[:, kt, :],
                                                 hT[:, kt, :],
                                                 start=(kt == 0), stop=(kt == 31))
                            tg = scr()
                            nc.vector.tensor_scalar_mul(tg[:], ps[:],
                                                        mod_ag[:, 24 + dc:25 + dc])
                            yrow = scr()
                            nc.vector.tensor_tensor(yrow[:], x2T[:, dc, :], tg[:],
                                                    OP.add)
                            nc.sync.dma_start(yT_r[:, dc, :], yrow[:])

    return nc


_NC_CACHE = None


def _prep_in_maps(inputs):
    x = np.asarray(inputs["x"], dtype=np.float32)
    c = np.asarray(inputs["c"], dtype=np.float32)
    cos = np.asarray(inputs["cos"], dtype=np.float32)
    sin = np.asarray(inputs["sin"], dtype=np.float32)

    def b16(a):
        return np.ascontiguousarray(a).astype(ml_dtypes.bfloat16)

    def f8(a, scale=1.0):
        return (np.ascontiguousarray(a) * scale).astype(ml_dtypes.float8_e4m3)

    wqkv_8 = f8(inputs["W_qkv"], WS)
    # dc-major permutations: [K, 8*128] -> [8_dc, 128_p, K/128_kt, 128_m]
    wout_8 = f8(np.asarray(inputs["W_out"], np.float32)
                .reshape(8, 128, 8, 128).transpose(2, 1, 0, 3).reshape(8, 128, 1024),
                WS)
    w1_b = b16(inputs["W1"])
    w2_b = b16(np.asarray(inputs["W2"], np.float32)
               .reshape(32, 128, 8, 128).transpose(2, 1, 0, 3).reshape(8, 128, 4096))
    wada_full = b16(inputs["W_ada"])
    bada_full = b16(np.asarray(inputs["b_ada"], dtype=np.float32).reshape(1, 12, 512))

    jj = np.arange(128) % 64
    pair = jj // 2
    sign = np.where(jj % 2 == 0, -1.0, 1.0).astype(np.float32)
    pswap_m = np.zeros((128, 128), np.float32)
    pswap_m[np.arange(128) ^ 1, np.arange(128)] = 1.0
    pswap_m = b16(pswap_m)

    in_maps = []
    for g in range(8):
        b, r = g // G, g % G
        rows = slice(r * SL, (r + 1) * SL)
        cl = cos[rows, 0:HD // 2]     # [512, 32]
        sl = sin[rows, 0:HD // 2]
        cosdT_m = b16(np.ascontiguousarray(cl.T[pair]))             # [128, 512]
        sindT_m = b16(np.ascontiguousarray(sl.T[pair] * sign[:, None]))
        in_maps.append({
            "xT": np.ascontiguousarray(x[b, rows, :].T),
            "cT": b16(c[b].reshape(8, 128).T),
            "bada_loc": np.ascontiguousarray(bada_full[:, 0:4, :]),
            "bada_sh": np.ascontiguousarray(bada_full[:, 4 + 2 * r:6 + 2 * r, :]),
            "cosdT": cosdT_m,
            "sindT": sindT_m,
            "pswap": pswap_m,
            "wqkv": wqkv_8, "wout": wout_8, "w1": w1_b, "w2": w2_b,
            "wada_loc": np.ascontiguousarray(wada_full[:, 0:2048]),
            "wada_sh": np.ascontiguousarray(
                wada_full[:, 2048 + 1024 * r:2048 + 1024 * (r + 1)]),
        })
    return in_maps


LAST_RESULT = None


def kernel(**inputs) -> np.ndarray:
    global _NC_CACHE, LAST_RESULT
    if _NC_CACHE is None:
        _NC_CACHE = build()
    nc = _NC_CACHE
    in_maps = _prep_in_maps(inputs)
    res = run_bass_kernel_spmd(nc, in_maps, core_ids=list(range(8)))
    LAST_RESULT = res
    y = np.empty((B, S, D), np.float32)
    for g in range(8):
        b, r = g // G, g % G
        y[b, r * SL:(r + 1) * SL, :] = res.results[g]["yT"].T
    return y
